# revision 1
# baseline (speedup 1.0000x reference)
"""Chunked non-uniform DFT on 8 Trainium2 NeuronCores (Bass/Tile).

vis[b,k] = sum_p exp(-2pi*i*(u_k*l_p + v_k*m_p + w_k*(n_p-1))) * sky[b,p]

Per core (visibilities sharded 8 ways => V_local = 2048):
  - t[p,k] = l_p*u_k + m_p*v_k + (n_p-1)*w_k computed on the Vector engine:
    u/v/w are replicated across all 128 partitions once (broadcast DMA);
    per pixel-chunk, l/m/n1 enter as per-partition scalars via
    tensor_scalar + 2x affine_then_add. (PE matmuls for this were slower
    here: this environment pays a large fixed cost per matmul instruction.)
  - r = t - round(t) in [-0.5, 0.5] via magic-number round + subtract,
    batched over chunk groups (large free dims, few instructions).
  - S = sin(2*pi*r), C = sin(pi/2 - 2*pi*|r|) = cos(2*pi*t) on ACT,
    written as fp16.
  - vis partial sums: PE matmuls, sky (4 columns: R0,R1,I0,I1, fp16) as
    stationary operand, S/C moving with N=1024, col-tiled to PE column
    groups 0 / 32 accumulating into PSUM partitions 0-3 / 32-35.
  - Host combines: vis_r[b] = C.R_b + S.I_b ; vis_i[b] = C.I_b - S.R_b.
"""

import numpy as np

B = 2
P = 16384
V = 16384
N_CORES = 8
VL = V // N_CORES  # 2048

MAGIC = float(1.5 * 2**23)
TWO_PI = float(2.0 * np.pi)
HALF_PI = float(0.5 * np.pi)

PIX_CHUNK = 128
N_PC = P // PIX_CHUNK   # 128
GROUP = 2               # pix-chunks per batched round/abs/ACT group
MM_N = 512             # stage-C matmul free dim (one PSUM bank)

_COMPILED = None


def _build(repeat=1):
    import concourse.bacc as bacc
    import concourse.mybir as mybir
    import concourse.tile as tile
    from concourse.alu_op_type import AluOpType

    nc = bacc.Bacc("TRN2", target_bir_lowering=False, debug=False,
                   num_devices=N_CORES)
    f32 = mybir.dt.float32
    f16 = mybir.dt.float16
    u32 = mybir.dt.uint32

    # lmn_cols[p, pc*3 + c]: coordinate c (l, m, n-1) of pixel (pc*128+p)
    lmn_d = nc.dram_tensor("lmnc", [PIX_CHUNK, N_PC * 3], f32,
                           kind="ExternalInput")
    uvw_d = nc.dram_tensor("uvw", [3, VL], f32, kind="ExternalInput")
    sky4_d = nc.dram_tensor("sky4", [PIX_CHUNK, N_PC * 4], f16,
                            kind="ExternalInput")
    out_d = nc.dram_tensor("out8", [8, VL], f32, kind="ExternalOutput")

    GFD = GROUP * VL

    with tile.TileContext(nc) as tc:
        with (
            tc.tile_pool(name="const", bufs=1) as constp,
            tc.tile_pool(name="inp", bufs=1) as inp,
            tc.tile_pool(name="tx", bufs=2) as txp,
            tc.tile_pool(name="ty", bufs=2) as typ,
            tc.tile_pool(name="rt", bufs=2) as rp,
            tc.tile_pool(name="rat", bufs=2) as rap,
            tc.tile_pool(name="st", bufs=2) as sp,
            tc.tile_pool(name="ct", bufs=2) as cp,
            tc.tile_pool(name="outs", bufs=1) as outp,
            tc.tile_pool(name="vps", bufs=1, space="PSUM") as vpsp,
        ):
            halfpi_t = constp.tile([128, 1], f32)
            nc.vector.memset(halfpi_t[:], HALF_PI)

            lmn_t = inp.tile([PIX_CHUNK, N_PC * 3], f32)
            nc.sync.dma_start(lmn_t[:], lmn_d[:])
            sky4_t = inp.tile([PIX_CHUNK, N_PC * 4], f16)
            nc.sync.dma_start(sky4_t[:], sky4_d[:])

            # u/v/w rows replicated across all 128 partitions
            reps = []
            for c in range(3):
                rep = inp.tile([128, VL], f32, tag=f"rep{c}")
                nc.sync.dma_start(rep[:], uvw_d[c:c + 1, :].to_broadcast(
                    (128, VL)))
                reps.append(rep)
            u_rep, v_rep, w_rep = reps

            vis_ps = vpsp.tile([36, VL], f32)

            for _rep in range(repeat):
              for g in range(N_PC // GROUP):
                t_x = txp.tile([128, GFD], f32)
                t_y = typ.tile([128, GFD], f32)
                r_t = rp.tile([128, GFD], f32)
                ra_t = rap.tile([128, GFD], f32)
                s_t = sp.tile([128, GFD], f16)
                c_t = cp.tile([128, GFD], f16)

                for h in range(GROUP):
                    pc = g * GROUP + h
                    sl = slice(h * VL, (h + 1) * VL)
                    l_col = lmn_t[:, pc * 3:pc * 3 + 1]
                    m_col = lmn_t[:, pc * 3 + 1:pc * 3 + 2]
                    n1_col = lmn_t[:, pc * 3 + 2:pc * 3 + 3]
                    # t = l*u
                    nc.vector.tensor_scalar(
                        t_x[:, sl], u_rep[:], l_col, None,
                        op0=AluOpType.mult)
                    # t += m*v ; t += n1*w
                    nc.vector.affine_then_add(
                        t_y[:, sl], v_rep[:], t_x[:, sl],
                        scale=m_col, bias=0.0)
                    nc.vector.affine_then_add(
                        t_x[:, sl], w_rep[:], t_y[:, sl],
                        scale=n1_col, bias=0.0)

                # k = round(t); r = t - k; ra = |r|
                nc.vector.tensor_scalar(
                    t_y[:], t_x[:], MAGIC, MAGIC,
                    op0=AluOpType.add, op1=AluOpType.subtract)
                nc.vector.tensor_tensor(
                    r_t[:], t_x[:], t_y[:], op=AluOpType.subtract)
                nc.vector.tensor_scalar(
                    ra_t[:].bitcast(u32), r_t[:].bitcast(u32),
                    0x7FFFFFFF, None, op0=AluOpType.bitwise_and)

                nc.scalar.activation(
                    s_t[:], r_t[:], mybir.ActivationFunctionType.Sin,
                    bias=0.0, scale=TWO_PI)
                nc.scalar.activation(
                    c_t[:], ra_t[:], mybir.ActivationFunctionType.Sin,
                    bias=halfpi_t[:], scale=-TWO_PI)

                for h in range(GROUP):
                    pc = g * GROUP + h
                    sky_sl = sky4_t[:, pc * 4:(pc + 1) * 4]
                    start = pc == 0
                    stop = pc == N_PC - 1
                    for n in range(VL // MM_N):
                        vsl = slice(h * VL + n * MM_N, h * VL + (n + 1) * MM_N)
                        osl = slice(n * MM_N, (n + 1) * MM_N)
                        nc.tensor.matmul(
                            vis_ps[0:4, osl], sky_sl, s_t[:, vsl],
                            start=start, stop=stop, tile_position=(0, 0))
                        nc.tensor.matmul(
                            vis_ps[32:36, osl], sky_sl, c_t[:, vsl],
                            start=start, stop=stop, tile_position=(0, 32))

            out_t = outp.tile([36, VL], f32)
            nc.scalar.copy(out_t[0:4, :], vis_ps[0:4, :])
            nc.scalar.copy(out_t[32:36, :], vis_ps[32:36, :])
            nc.sync.dma_start(out_d[0:4, :], out_t[0:4, :])
            nc.sync.dma_start(out_d[4:8, :], out_t[32:36, :])

    nc.compile()
    return nc


def _prep_inputs(sky_real, sky_imag, l_coords, m_coords, n_coords,
                 u_coords, v_coords, w_coords):
    # lmn_cols[p, pc*3+c]
    lmn = np.stack([l_coords, m_coords, n_coords - 1.0], axis=1)  # [P, 3]
    lmn = lmn.reshape(N_PC, PIX_CHUNK, 3).transpose(1, 0, 2).reshape(
        PIX_CHUNK, N_PC * 3).astype(np.float32)
    lmn = np.ascontiguousarray(lmn)

    sky4 = np.stack([sky_real[0], sky_real[1], sky_imag[0], sky_imag[1]],
                    axis=1)                                       # [P, 4]
    sky4 = sky4.reshape(N_PC, PIX_CHUNK, 4).transpose(1, 0, 2).reshape(
        PIX_CHUNK, N_PC * 4).astype(np.float16)
    sky4 = np.ascontiguousarray(sky4)

    in_maps = []
    for c in range(N_CORES):
        sl = slice(c * VL, (c + 1) * VL)
        uvw = np.ascontiguousarray(
            np.stack([u_coords[sl], v_coords[sl], w_coords[sl]])
            .astype(np.float32))
        in_maps.append({"lmnc": lmn, "uvw": uvw, "sky4": sky4})
    return in_maps


def kernel(sky_real, sky_imag, l_coords, m_coords, n_coords,
           u_coords, v_coords, w_coords):
    global _COMPILED
    from concourse.bass_utils import run_bass_kernel_spmd

    if _COMPILED is None:
        _COMPILED = _build()
    nc = _COMPILED

    in_maps = _prep_inputs(sky_real, sky_imag, l_coords, m_coords, n_coords,
                           u_coords, v_coords, w_coords)
    res = run_bass_kernel_spmd(nc, in_maps, core_ids=list(range(N_CORES)))

    vis = np.empty((B, V), dtype=np.complex64)
    for c in range(N_CORES):
        sl = slice(c * VL, (c + 1) * VL)
        o = res.results[c]["out8"]  # SR0, SR1, SI0, SI1, CR0, CR1, CI0, CI1
        sr0, sr1, si0, si1, cr0, cr1, ci0, ci1 = o
        vis[0, sl] = (cr0 + si0) + 1j * (ci0 - sr0)
        vis[1, sl] = (cr1 + si1) + 1j * (ci1 - sr1)
    return vis



# revision 3
# speedup vs baseline: 2.6885x; 2.6885x over previous
"""Chunked non-uniform DFT on 8 Trainium2 NeuronCores (Bass/Tile).

vis[b,k] = sum_p exp(-2pi*i*(u_k*l_p + v_k*m_p + w_k*(n_p-1))) * sky[b,p]

Per core (visibilities sharded 8 ways => V_local = 2048):
  - t[p,k] = l_p*u_k + m_p*v_k + (n_p-1)*w_k computed on the Vector engine:
    u/v/w are replicated across all 128 partitions once (broadcast DMA);
    per pixel-chunk, l/m/n1 enter as per-partition scalars via
    tensor_scalar + 2x affine_then_add. (PE matmuls for this were slower
    here: this environment pays a large fixed cost per matmul instruction.)
  - r = t - round(t) in [-0.5, 0.5] via magic-number round + subtract,
    batched over chunk groups (large free dims, few instructions).
  - S = sin(2*pi*r), C = sin(pi/2 - 2*pi*|r|) = cos(2*pi*t) on ACT,
    written as fp16.
  - vis partial sums: PE matmuls, sky (4 columns: R0,R1,I0,I1, fp16) as
    stationary operand, S/C moving with N=1024, col-tiled to PE column
    groups 0 / 32 accumulating into PSUM partitions 0-3 / 32-35.
  - Host combines: vis_r[b] = C.R_b + S.I_b ; vis_i[b] = C.I_b - S.R_b.
"""

import numpy as np

B = 2
P = 16384
V = 16384
N_CORES = 8
VL = V // N_CORES  # 2048

MAGIC = float(1.5 * 2**23)
TWO_PI = float(2.0 * np.pi)
HALF_PI = float(0.5 * np.pi)

PIX_CHUNK = 128
N_PC = P // PIX_CHUNK   # 128
GROUP = 2               # pix-chunks per batched round/abs/ACT group
MM_N = 512             # stage-C matmul free dim (one PSUM bank)

_COMPILED = None


def _build(repeat=1):
    import concourse.bacc as bacc
    import concourse.mybir as mybir
    import concourse.tile as tile
    from concourse.alu_op_type import AluOpType

    nc = bacc.Bacc("TRN2", target_bir_lowering=False, debug=False,
                   num_devices=N_CORES)
    f32 = mybir.dt.float32
    f16 = mybir.dt.float16
    u32 = mybir.dt.uint32

    # lmn_cols[p, pc*3 + c]: coordinate c (l, m, n-1) of pixel (pc*128+p)
    lmn_d = nc.dram_tensor("lmnc", [PIX_CHUNK, N_PC * 3], f32,
                           kind="ExternalInput")
    uvw_d = nc.dram_tensor("uvw", [3, VL], f32, kind="ExternalInput")
    sky4_d = nc.dram_tensor("sky4", [PIX_CHUNK, N_PC * 4], f16,
                            kind="ExternalInput")
    out_d = nc.dram_tensor("out8", [8, VL], f32, kind="ExternalOutput")

    GFD = GROUP * VL

    with tile.TileContext(nc) as tc:
        with (
            tc.tile_pool(name="const", bufs=1) as constp,
            tc.tile_pool(name="inp", bufs=1) as inp,
            tc.tile_pool(name="tx", bufs=2) as txp,
            tc.tile_pool(name="ty", bufs=2) as typ,
            tc.tile_pool(name="rt", bufs=2) as rp,
            tc.tile_pool(name="rat", bufs=2) as rap,
            tc.tile_pool(name="st", bufs=2) as sp,
            tc.tile_pool(name="ct", bufs=2) as cp,
            tc.tile_pool(name="outs", bufs=1) as outp,
            tc.tile_pool(name="vps", bufs=1, space="PSUM") as vpsp,
        ):
            halfpi_t = constp.tile([128, 1], f32)
            nc.vector.memset(halfpi_t[:], HALF_PI)

            lmn_t = inp.tile([PIX_CHUNK, N_PC * 3], f32)
            nc.sync.dma_start(lmn_t[:], lmn_d[:])
            sky4_t = inp.tile([PIX_CHUNK, N_PC * 4], f16)
            nc.sync.dma_start(sky4_t[:], sky4_d[:])

            # u/v/w rows replicated across all 128 partitions
            reps = []
            for c in range(3):
                rep = inp.tile([128, VL], f32, tag=f"rep{c}")
                nc.sync.dma_start(rep[:], uvw_d[c:c + 1, :].to_broadcast(
                    (128, VL)))
                reps.append(rep)
            u_rep, v_rep, w_rep = reps

            vis_ps = vpsp.tile([36, VL], f32)

            for _rep in range(repeat):
              for g in range(N_PC // GROUP):
                t_x = txp.tile([128, GFD], f32)
                t_y = typ.tile([128, GFD], f32)
                r_t = rp.tile([128, GFD], f32)
                ra_t = rap.tile([128, GFD], f32)
                s_t = sp.tile([128, GFD], f16)
                c_t = cp.tile([128, GFD], f16)

                for h in range(GROUP):
                    pc = g * GROUP + h
                    sl = slice(h * VL, (h + 1) * VL)
                    l_col = lmn_t[:, pc * 3:pc * 3 + 1]
                    m_col = lmn_t[:, pc * 3 + 1:pc * 3 + 2]
                    n1_col = lmn_t[:, pc * 3 + 2:pc * 3 + 3]
                    # t = l*u
                    nc.vector.tensor_scalar(
                        t_x[:, sl], u_rep[:], l_col, None,
                        op0=AluOpType.mult)
                    # t += m*v ; t += n1*w
                    nc.vector.affine_then_add(
                        t_y[:, sl], v_rep[:], t_x[:, sl],
                        scale=m_col, bias=0.0)
                    nc.vector.affine_then_add(
                        t_x[:, sl], w_rep[:], t_y[:, sl],
                        scale=n1_col, bias=0.0)

                # k = round(t); r = t - k; ra = |r|
                nc.vector.tensor_scalar(
                    t_y[:], t_x[:], MAGIC, MAGIC,
                    op0=AluOpType.add, op1=AluOpType.subtract)
                nc.vector.tensor_tensor(
                    r_t[:], t_x[:], t_y[:], op=AluOpType.subtract)
                nc.vector.tensor_scalar(
                    ra_t[:].bitcast(u32), r_t[:].bitcast(u32),
                    0x7FFFFFFF, None, op0=AluOpType.bitwise_and)

                nc.scalar.activation(
                    s_t[:], r_t[:], mybir.ActivationFunctionType.Sin,
                    bias=0.0, scale=TWO_PI)
                nc.scalar.activation(
                    c_t[:], ra_t[:], mybir.ActivationFunctionType.Sin,
                    bias=halfpi_t[:], scale=-TWO_PI)

                for h in range(GROUP):
                    pc = g * GROUP + h
                    sky_sl = sky4_t[:, pc * 4:(pc + 1) * 4]
                    start = pc == 0
                    stop = pc == N_PC - 1
                    for n in range(VL // MM_N):
                        vsl = slice(h * VL + n * MM_N, h * VL + (n + 1) * MM_N)
                        osl = slice(n * MM_N, (n + 1) * MM_N)
                        nc.tensor.matmul(
                            vis_ps[0:4, osl], sky_sl, s_t[:, vsl],
                            start=start, stop=stop, tile_position=(0, 0))
                        nc.tensor.matmul(
                            vis_ps[32:36, osl], sky_sl, c_t[:, vsl],
                            start=start, stop=stop, tile_position=(0, 32))

            out_t = outp.tile([36, VL], f32)
            nc.scalar.copy(out_t[0:4, :], vis_ps[0:4, :])
            nc.scalar.copy(out_t[32:36, :], vis_ps[32:36, :])
            nc.sync.dma_start(out_d[0:4, :], out_t[0:4, :])
            nc.sync.dma_start(out_d[4:8, :], out_t[32:36, :])

    nc.compile()
    return nc


def _prep_inputs(sky_real, sky_imag, l_coords, m_coords, n_coords,
                 u_coords, v_coords, w_coords):
    # lmn_cols[p, pc*3+c]
    lmn = np.stack([l_coords, m_coords, n_coords - 1.0], axis=1)  # [P, 3]
    lmn = lmn.reshape(N_PC, PIX_CHUNK, 3).transpose(1, 0, 2).reshape(
        PIX_CHUNK, N_PC * 3).astype(np.float32)
    lmn = np.ascontiguousarray(lmn)

    sky4 = np.stack([sky_real[0], sky_real[1], sky_imag[0], sky_imag[1]],
                    axis=1)                                       # [P, 4]
    sky4 = sky4.reshape(N_PC, PIX_CHUNK, 4).transpose(1, 0, 2).reshape(
        PIX_CHUNK, N_PC * 4).astype(np.float16)
    sky4 = np.ascontiguousarray(sky4)

    in_maps = []
    for c in range(N_CORES):
        sl = slice(c * VL, (c + 1) * VL)
        uvw = np.ascontiguousarray(
            np.stack([u_coords[sl], v_coords[sl], w_coords[sl]])
            .astype(np.float32))
        in_maps.append({"lmnc": lmn, "uvw": uvw, "sky4": sky4})
    return in_maps


# Cached dispatch: run_bass_kernel_spmd (axon path -> run_bass_via_pjrt)
# rebuilds the jax.jit(shard_map(...)) closure on EVERY call, retracing and
# relowering the whole program each time -- hundreds of ms of pure host
# overhead per call. Build the jitted executable once and reuse it.
_RUNNER = None


def _get_runner():
    global _RUNNER
    if _RUNNER is not None:
        return _RUNNER

    import jax
    import concourse.mybir as mybir
    from concourse import bass2jax
    from jax.experimental.shard_map import shard_map
    from jax.sharding import Mesh, PartitionSpec

    nc = _build()
    bass2jax.install_neuronx_cc_hook()
    assert nc.dbg_addr is None
    part_name = (nc.partition_id_tensor.name
                 if nc.partition_id_tensor else None)

    in_names, out_names, out_avals = [], [], []
    for alloc in nc.m.functions[0].allocations:
        if not isinstance(alloc, mybir.MemoryLocationSet):
            continue
        name = alloc.memorylocations[0].name
        if alloc.kind == "ExternalInput":
            if name != part_name:
                in_names.append(name)
        elif alloc.kind == "ExternalOutput":
            shape = tuple(alloc.tensor_shape)
            dtype = mybir.dt.np(alloc.dtype)
            out_names.append(name)
            out_avals.append(jax.core.ShapedArray(shape, dtype))
    n_params = len(in_names)
    all_names = list(in_names) + list(out_names)
    if part_name is not None:
        all_names.append(part_name)
    all_names = tuple(all_names)

    def _body(*args):
        operands = list(args)
        if part_name is not None:
            operands.append(bass2jax.partition_id_tensor())
        outs = bass2jax._bass_exec_p.bind(
            *operands,
            out_avals=tuple(out_avals),
            in_names=all_names,
            out_names=tuple(out_names),
            lowering_input_output_aliases=(),
            sim_require_finite=True,
            sim_require_nnan=True,
            nc=nc,
        )
        return tuple(outs)

    devices = jax.devices()[:N_CORES]
    mesh = Mesh(np.asarray(devices), ("core",))
    n_outs = len(out_names)
    sharded = jax.jit(
        shard_map(_body, mesh=mesh,
                  in_specs=(PartitionSpec("core"),) * (n_params + n_outs),
                  out_specs=(PartitionSpec("core"),) * n_outs,
                  check_rep=False),
        donate_argnums=tuple(range(n_params, n_params + n_outs)),
        keep_unused=True,
    )
    zero_shapes = [((N_CORES * a.shape[0],) + tuple(a.shape[1:]), a.dtype)
                   for a in out_avals]
    _RUNNER = (sharded, in_names, out_names, out_avals, zero_shapes)
    return _RUNNER


def kernel(sky_real, sky_imag, l_coords, m_coords, n_coords,
           u_coords, v_coords, w_coords):
    sharded, in_names, out_names, out_avals, zero_shapes = _get_runner()

    in_maps = _prep_inputs(sky_real, sky_imag, l_coords, m_coords, n_coords,
                           u_coords, v_coords, w_coords)
    concat_in = [
        np.concatenate([in_maps[c][name] for c in range(N_CORES)], axis=0)
        for name in in_names
    ]
    concat_zeros = [np.zeros(s, d) for s, d in zero_shapes]
    out_arrs = sharded(*concat_in, *concat_zeros)
    o8 = np.asarray(out_arrs[out_names.index("out8")]).reshape(
        N_CORES, 8, VL)

    vis = np.empty((B, V), dtype=np.complex64)
    for c in range(N_CORES):
        sl = slice(c * VL, (c + 1) * VL)
        sr0, sr1, si0, si1, cr0, cr1, ci0, ci1 = o8[c]
        vis[0, sl] = (cr0 + si0) + 1j * (ci0 - sr0)
        vis[1, sl] = (cr1 + si1) + 1j * (ci1 - sr1)
    return vis



# revision 9
# speedup vs baseline: 6.1767x; 2.2975x over previous
"""Chunked non-uniform DFT on 8 Trainium2 NeuronCores (Bass/Tile).

vis[b,k] = sum_p exp(-2pi*i*(u_k*l_p + v_k*m_p + w_k*(n_p-1))) * sky[b,p]

The wall-clock through the axon PJRT tunnel is dominated by a fixed
round-trip (~60ms) plus wire bytes at ~40MB/s, so the kernel minimizes
host<->device traffic:
  - Each core uploads only 1/8 of the pixel arrays (its l/m blocks and
    sky quadrature columns) plus its own uvw slice (~56KB/core); the
    full pixel arrays are reassembled on-device with a DRAM AllGather
    over NeuronLink.
  - n-1 is recomputed on device from l,m (n = sqrt(1-l^2-m^2)), saving
    one upload array.
  - Outputs downloaded as fp16 [8, VL] (32KB/core); host combines
    visR_b = C.R_b + S.I_b ; visI_b = C.I_b - S.R_b.
  - No donated zero output buffers (kernel fully writes its output).
  - The jitted shard_map executable is built once and cached; the stock
    run_bass_kernel_spmd re-jits on every call.

Compute per core (visibilities sharded 8 ways => V_local = 2048):
  - t[p,k] = l_p*u_k + m_p*v_k + (n_p-1)*w_k on the Vector engine:
    u/v/w replicated across 128 partitions once (broadcast DMA); per
    pixel-chunk, l/m/n1 enter as per-partition scalars via
    tensor_scalar + 2x affine_then_add.
  - r = t - round(t) in [-0.5, 0.5] via magic-number round + subtract.
  - S = sin(2*pi*r), C = sin(pi/2 - 2*pi*|r|) = cos(2*pi*t) on ACT, fp16.
  - PE matmuls: sky (4 columns R0,R1,I0,I1 fp16) stationary, S/C moving,
    col groups 0 / 32 accumulating into PSUM rows 0-3 / 32-35.
"""

import numpy as np

B = 2
P = 16384
V = 16384
N_CORES = 8
VL = V // N_CORES  # 2048

MAGIC = float(1.5 * 2**23)
TWO_PI = float(2.0 * np.pi)
HALF_PI = float(0.5 * np.pi)

PIX_CHUNK = 128
N_PC = P // PIX_CHUNK        # 128 pixel chunks total
PCL = N_PC // N_CORES        # 16 pixel chunks owned per core
GROUP = 2                    # pix-chunks per batched round/abs/ACT group
MM_N = 512                   # stage-C matmul free dim (one PSUM bank)

RG = [list(range(N_CORES))]


def _build(repeat=1):
    import concourse.bacc as bacc
    import concourse.mybir as mybir
    import concourse.tile as tile
    from concourse.alu_op_type import AluOpType

    nc = bacc.Bacc("TRN2", target_bir_lowering=False, debug=False,
                   num_devices=N_CORES)
    f32 = mybir.dt.float32
    f16 = mybir.dt.float16
    u32 = mybir.dt.uint32

    # Per-core shards: l/m blocks for this core's PCL pixel chunks,
    # sky quadrature (R0,R1,I0,I1 per chunk), own uvw slice.
    lm_d = nc.dram_tensor("lm", [PIX_CHUNK, PCL * 2], f32,
                          kind="ExternalInput")
    sky_d = nc.dram_tensor("sky4", [PIX_CHUNK, PCL * 4], f16,
                           kind="ExternalInput")
    uvw_d = nc.dram_tensor("uvw", [3, VL], f32, kind="ExternalInput")
    out_d = nc.dram_tensor("out8", [8, VL], f16, kind="ExternalOutput")

    GFD = GROUP * VL

    with tile.TileContext(nc) as tc:
        with (
            tc.tile_pool(name="dram", bufs=1, space="DRAM") as dramp,
            tc.tile_pool(name="const", bufs=1) as constp,
            tc.tile_pool(name="inp", bufs=1) as inp,
            tc.tile_pool(name="vps", bufs=1, space="PSUM") as vpsp,
        ):
            # ---- gather the pixel arrays from all cores over NeuronLink
            lm_bnc = dramp.tile([PIX_CHUNK, PCL * 2], f32)
            sky_bnc = dramp.tile([PIX_CHUNK, PCL * 4], f16)
            lm_g = dramp.tile([N_CORES * PIX_CHUNK, PCL * 2], f32,
                              addr_space="Shared")
            sky_g = dramp.tile([N_CORES * PIX_CHUNK, PCL * 4], f16,
                               addr_space="Shared")
            nc.gpsimd.dma_start(lm_bnc[:], lm_d[:])
            nc.gpsimd.dma_start(sky_bnc[:], sky_d[:])
            nc.gpsimd.collective_compute(
                "AllGather", mybir.AluOpType.bypass, replica_groups=RG,
                ins=[lm_bnc.opt()], outs=[lm_g.opt()])
            nc.gpsimd.collective_compute(
                "AllGather", mybir.AluOpType.bypass, replica_groups=RG,
                ins=[sky_bnc.opt()], outs=[sky_g.opt()])

            halfpi_t = constp.tile([128, 1], f32)
            nc.vector.memset(halfpi_t[:], HALF_PI)

            # l_t/m_t: [128, N_PC], col = global pixel chunk (core-major)
            l_t = inp.tile([PIX_CHUNK, N_PC], f32)
            m_t = inp.tile([PIX_CHUNK, N_PC], f32)
            sky4_t = inp.tile([PIX_CHUNK, N_PC * 4], f16)
            for c in range(N_CORES):
                rows = slice(c * PIX_CHUNK, (c + 1) * PIX_CHUNK)
                nc.sync.dma_start(l_t[:, c * PCL:(c + 1) * PCL],
                                  lm_g[rows, 0:PCL])
                nc.sync.dma_start(m_t[:, c * PCL:(c + 1) * PCL],
                                  lm_g[rows, PCL:2 * PCL])
                nc.sync.dma_start(sky4_t[:, c * PCL * 4:(c + 1) * PCL * 4],
                                  sky_g[rows, :])

            # n1 = sqrt(1 - l^2 - m^2) - 1
            n1_t = inp.tile([PIX_CHUNK, N_PC], f32)
            sq_t = inp.tile([PIX_CHUNK, N_PC], f32)
            sq2_t = inp.tile([PIX_CHUNK, N_PC], f32)
            nc.vector.tensor_tensor(sq_t[:], l_t[:], l_t[:],
                                    op=AluOpType.mult)
            nc.vector.tensor_tensor(sq2_t[:], m_t[:], m_t[:],
                                    op=AluOpType.mult)
            nc.vector.tensor_tensor(sq_t[:], sq_t[:], sq2_t[:],
                                    op=AluOpType.add)
            nc.scalar.activation(
                sq2_t[:], sq_t[:], mybir.ActivationFunctionType.Sqrt,
                bias=1.0, scale=-1.0)
            nc.vector.tensor_scalar(
                n1_t[:], sq2_t[:], -1.0, None, op0=AluOpType.add)

            # u/v/w rows replicated across all 128 partitions
            reps = []
            for c in range(3):
                rep = inp.tile([128, VL], f32, tag=f"rep{c}")
                nc.sync.dma_start(rep[:], uvw_d[c:c + 1, :].to_broadcast(
                    (128, VL)))
                reps.append(rep)
            u_rep, v_rep, w_rep = reps

            vis_ps = vpsp.tile([36, VL], f32)

            with (
                tc.tile_pool(name="tx", bufs=2) as txp,
                tc.tile_pool(name="ty", bufs=2) as typ,
                tc.tile_pool(name="rt", bufs=2) as rp,
                tc.tile_pool(name="rat", bufs=2) as rap,
                tc.tile_pool(name="st", bufs=2) as sp,
                tc.tile_pool(name="ct", bufs=2) as cp,
            ):
             for _rep in range(repeat):
              for g in range(N_PC // GROUP):
                t_x = txp.tile([128, GFD], f32)
                t_y = typ.tile([128, GFD], f32)
                r_t = rp.tile([128, GFD], f32)
                ra_t = rap.tile([128, GFD], f32)
                s_t = sp.tile([128, GFD], f16)
                c_t = cp.tile([128, GFD], f16)

                for h in range(GROUP):
                    pc = g * GROUP + h
                    sl = slice(h * VL, (h + 1) * VL)
                    l_col = l_t[:, pc:pc + 1]
                    m_col = m_t[:, pc:pc + 1]
                    n1_col = n1_t[:, pc:pc + 1]
                    # t = l*u
                    nc.vector.tensor_scalar(
                        t_x[:, sl], u_rep[:], l_col, None,
                        op0=AluOpType.mult)
                    # t += m*v ; t += n1*w
                    nc.vector.affine_then_add(
                        t_y[:, sl], v_rep[:], t_x[:, sl],
                        scale=m_col, bias=0.0)
                    nc.vector.affine_then_add(
                        t_x[:, sl], w_rep[:], t_y[:, sl],
                        scale=n1_col, bias=0.0)

                # k = round(t); r = t - k; ra = |r|
                nc.vector.tensor_scalar(
                    t_y[:], t_x[:], MAGIC, MAGIC,
                    op0=AluOpType.add, op1=AluOpType.subtract)
                nc.vector.tensor_tensor(
                    r_t[:], t_x[:], t_y[:], op=AluOpType.subtract)
                nc.vector.tensor_scalar(
                    ra_t[:].bitcast(u32), r_t[:].bitcast(u32),
                    0x7FFFFFFF, None, op0=AluOpType.bitwise_and)

                nc.scalar.activation(
                    s_t[:], r_t[:], mybir.ActivationFunctionType.Sin,
                    bias=0.0, scale=TWO_PI)
                nc.scalar.activation(
                    c_t[:], ra_t[:], mybir.ActivationFunctionType.Sin,
                    bias=halfpi_t[:], scale=-TWO_PI)

                for h in range(GROUP):
                    pc = g * GROUP + h
                    sky_sl = sky4_t[:, pc * 4:(pc + 1) * 4]
                    start = pc == 0
                    stop = pc == N_PC - 1
                    for n in range(VL // MM_N):
                        vsl = slice(h * VL + n * MM_N, h * VL + (n + 1) * MM_N)
                        osl = slice(n * MM_N, (n + 1) * MM_N)
                        nc.tensor.matmul(
                            vis_ps[0:4, osl], sky_sl, s_t[:, vsl],
                            start=start, stop=stop, tile_position=(0, 0))
                        nc.tensor.matmul(
                            vis_ps[32:36, osl], sky_sl, c_t[:, vsl],
                            start=start, stop=stop, tile_position=(0, 32))

            # PSUM rows 0:4 = [SR0,SR1,SI0,SI1], rows 32:36 = [CR0,...,CI1]
            # fp16 download; host combines (engine ops must start at a
            # quadrant-aligned partition base, which rules out an on-device
            # cross-partition combine without extra staging).
            with tc.tile_pool(name="cmb", bufs=1) as cmbp:
                out_t = cmbp.tile([36, VL], f16)
                nc.scalar.copy(out_t[0:4, :], vis_ps[0:4, :])
                nc.scalar.copy(out_t[32:36, :], vis_ps[32:36, :])
                nc.sync.dma_start(out_d[0:4, :], out_t[0:4, :])
                nc.sync.dma_start(out_d[4:8, :], out_t[32:36, :])

    nc.compile()
    return nc


def _prep_inputs(sky_real, sky_imag, l_coords, m_coords, n_coords,
                 u_coords, v_coords, w_coords):
    # [N_PC, 128] views: chunk pc covers pixels pc*128 .. pc*128+127
    lc = l_coords.reshape(N_PC, PIX_CHUNK).astype(np.float32)
    mc = m_coords.reshape(N_PC, PIX_CHUNK).astype(np.float32)

    sky4 = np.stack([sky_real[0], sky_real[1], sky_imag[0], sky_imag[1]],
                    axis=1)                                       # [P, 4]
    sky4 = sky4.reshape(N_PC, PIX_CHUNK, 4).astype(np.float16)

    in_maps = []
    for c in range(N_CORES):
        pcs = slice(c * PCL, (c + 1) * PCL)
        # lm: [128, PCL(l) | PCL(m)]
        lm = np.concatenate([lc[pcs].T, mc[pcs].T], axis=1)
        lm = np.ascontiguousarray(lm, dtype=np.float32)
        # sky: [128, PCL*4], col j*4+k = chunk (c*PCL+j), quadrature k
        s4 = np.ascontiguousarray(
            sky4[pcs].transpose(1, 0, 2).reshape(PIX_CHUNK, PCL * 4))
        sl = slice(c * VL, (c + 1) * VL)
        uvw = np.ascontiguousarray(
            np.stack([u_coords[sl], v_coords[sl], w_coords[sl]])
            .astype(np.float32))
        in_maps.append({"lm": lm, "sky4": s4, "uvw": uvw})
    return in_maps


# Cached dispatch: run_bass_kernel_spmd (axon path -> run_bass_via_pjrt)
# rebuilds the jax.jit(shard_map(...)) closure on EVERY call, retracing and
# relowering the whole program each time -- hundreds of ms of pure host
# overhead per call. Build the jitted executable once and reuse it.
_RUNNER = None


def _get_runner():
    global _RUNNER
    if _RUNNER is not None:
        return _RUNNER

    import jax
    import concourse.mybir as mybir
    from concourse import bass2jax
    from jax.experimental.shard_map import shard_map
    from jax.sharding import Mesh, PartitionSpec

    nc = _build()
    bass2jax.install_neuronx_cc_hook()
    assert nc.dbg_addr is None
    part_name = (nc.partition_id_tensor.name
                 if nc.partition_id_tensor else None)

    in_names, out_names, out_avals = [], [], []
    for alloc in nc.m.functions[0].allocations:
        if not isinstance(alloc, mybir.MemoryLocationSet):
            continue
        name = alloc.memorylocations[0].name
        if alloc.kind == "ExternalInput":
            if name != part_name:
                in_names.append(name)
        elif alloc.kind == "ExternalOutput":
            shape = tuple(alloc.tensor_shape)
            dtype = mybir.dt.np(alloc.dtype)
            out_names.append(name)
            out_avals.append(jax.core.ShapedArray(shape, dtype))
    n_params = len(in_names)
    # Outputs are fully written by the kernel, so no donated zero output
    # buffers are passed (saves their upload).
    all_names = list(in_names)
    if part_name is not None:
        all_names.append(part_name)
    all_names = tuple(all_names)

    def _body(*args):
        operands = list(args)
        if part_name is not None:
            operands.append(bass2jax.partition_id_tensor())
        outs = bass2jax._bass_exec_p.bind(
            *operands,
            out_avals=tuple(out_avals),
            in_names=all_names,
            out_names=tuple(out_names),
            lowering_input_output_aliases=(),
            sim_require_finite=True,
            sim_require_nnan=True,
            nc=nc,
        )
        return tuple(outs)

    devices = jax.devices()[:N_CORES]
    mesh = Mesh(np.asarray(devices), ("core",))
    n_outs = len(out_names)
    sharded = jax.jit(
        shard_map(_body, mesh=mesh,
                  in_specs=(PartitionSpec("core"),) * n_params,
                  out_specs=(PartitionSpec("core"),) * n_outs,
                  check_rep=False),
        keep_unused=True,
    )
    _RUNNER = (sharded, in_names, out_names, out_avals)
    return _RUNNER


def kernel(sky_real, sky_imag, l_coords, m_coords, n_coords,
           u_coords, v_coords, w_coords):
    sharded, in_names, out_names, out_avals = _get_runner()

    in_maps = _prep_inputs(sky_real, sky_imag, l_coords, m_coords, n_coords,
                           u_coords, v_coords, w_coords)
    concat_in = [
        np.concatenate([in_maps[c][name] for c in range(N_CORES)], axis=0)
        for name in in_names
    ]
    out_arrs = sharded(*concat_in)
    o8 = np.asarray(out_arrs[out_names.index("out8")]).astype(np.float32)
    o8 = o8.reshape(N_CORES, 8, VL)

    vis = np.empty((B, V), dtype=np.complex64)
    for c in range(N_CORES):
        sl = slice(c * VL, (c + 1) * VL)
        sr0, sr1, si0, si1, cr0, cr1, ci0, ci1 = o8[c]
        vis[0, sl] = (cr0 + si0) + 1j * (ci0 - sr0)
        vis[1, sl] = (cr1 + si1) + 1j * (ci1 - sr1)
    return vis


# revision 10
# speedup vs baseline: 6.7208x; 1.0881x over previous
"""Chunked non-uniform DFT on 8 Trainium2 NeuronCores (Bass/Tile).

vis[b,k] = sum_p exp(-2pi*i*(u_k*l_p + v_k*m_p + w_k*(n_p-1))) * sky[b,p]

The wall-clock through the axon PJRT tunnel is dominated by a fixed
round-trip (~60ms) plus wire bytes at ~40MB/s, so the kernel minimizes
host<->device traffic:
  - Each core uploads only 1/8 of the pixel arrays (its l/m blocks and
    sky quadrature columns) plus its own uvw slice (~56KB/core); the
    full pixel arrays are reassembled on-device with a DRAM AllGather
    over NeuronLink.
  - n-1 is recomputed on device from l,m (n = sqrt(1-l^2-m^2)), saving
    one upload array.
  - Partial sums are combined on device (visR_b = C.R_b + S.I_b,
    visI_b = C.I_b - S.R_b) and downloaded as fp16 [4, VL] (16KB/core).
  - No donated zero output buffers (kernel fully writes its output).
  - The jitted shard_map executable is built once and cached; the stock
    run_bass_kernel_spmd re-jits on every call.

Compute per core (visibilities sharded 8 ways => V_local = 2048):
  - t[p,k] = l_p*u_k + m_p*v_k + (n_p-1)*w_k on the Vector engine:
    u/v/w replicated across 128 partitions once (broadcast DMA); per
    pixel-chunk, l/m/n1 enter as per-partition scalars via
    tensor_scalar + 2x affine_then_add.
  - r = t - round(t) in [-0.5, 0.5] via magic-number round + subtract.
  - S = sin(2*pi*r), C = sin(pi/2 - 2*pi*|r|) = cos(2*pi*t) on ACT, fp16.
  - PE matmuls: sky (4 columns R0,R1,I0,I1 fp16) stationary, S/C moving,
    col groups 0 / 32 accumulating into PSUM rows 0-3 / 32-35.
"""

import numpy as np

B = 2
P = 16384
V = 16384
N_CORES = 8
VL = V // N_CORES  # 2048

MAGIC = float(1.5 * 2**23)
TWO_PI = float(2.0 * np.pi)
HALF_PI = float(0.5 * np.pi)

PIX_CHUNK = 128
N_PC = P // PIX_CHUNK        # 128 pixel chunks total
PCL = N_PC // N_CORES        # 16 pixel chunks owned per core
GROUP = 2                    # pix-chunks per batched round/abs/ACT group
MM_N = 512                   # stage-C matmul free dim (one PSUM bank)

RG = [list(range(N_CORES))]


def _build(repeat=1):
    import concourse.bacc as bacc
    import concourse.mybir as mybir
    import concourse.tile as tile
    from concourse.alu_op_type import AluOpType

    nc = bacc.Bacc("TRN2", target_bir_lowering=False, debug=False,
                   num_devices=N_CORES)
    f32 = mybir.dt.float32
    f16 = mybir.dt.float16
    u32 = mybir.dt.uint32

    # Per-core shards: l/m blocks for this core's PCL pixel chunks,
    # sky quadrature (R0,R1,I0,I1 per chunk), own uvw slice.
    lm_d = nc.dram_tensor("lm", [PIX_CHUNK, PCL * 2], f32,
                          kind="ExternalInput")
    sky_d = nc.dram_tensor("sky4", [PIX_CHUNK, PCL * 4], f16,
                           kind="ExternalInput")
    uvw_d = nc.dram_tensor("uvw", [3, VL], f32, kind="ExternalInput")
    out_d = nc.dram_tensor("out4", [4, VL], f16, kind="ExternalOutput")

    GFD = GROUP * VL

    with tile.TileContext(nc) as tc:
        with (
            tc.tile_pool(name="dram", bufs=1, space="DRAM") as dramp,
            tc.tile_pool(name="const", bufs=1) as constp,
            tc.tile_pool(name="inp", bufs=1) as inp,
            tc.tile_pool(name="vps", bufs=1, space="PSUM") as vpsp,
        ):
            # ---- gather the pixel arrays from all cores over NeuronLink
            lm_bnc = dramp.tile([PIX_CHUNK, PCL * 2], f32)
            sky_bnc = dramp.tile([PIX_CHUNK, PCL * 4], f16)
            lm_g = dramp.tile([N_CORES * PIX_CHUNK, PCL * 2], f32,
                              addr_space="Shared")
            sky_g = dramp.tile([N_CORES * PIX_CHUNK, PCL * 4], f16,
                               addr_space="Shared")
            nc.gpsimd.dma_start(lm_bnc[:], lm_d[:])
            nc.gpsimd.dma_start(sky_bnc[:], sky_d[:])
            nc.gpsimd.collective_compute(
                "AllGather", mybir.AluOpType.bypass, replica_groups=RG,
                ins=[lm_bnc.opt()], outs=[lm_g.opt()])
            nc.gpsimd.collective_compute(
                "AllGather", mybir.AluOpType.bypass, replica_groups=RG,
                ins=[sky_bnc.opt()], outs=[sky_g.opt()])

            halfpi_t = constp.tile([128, 1], f32)
            nc.vector.memset(halfpi_t[:], HALF_PI)

            # l_t/m_t: [128, N_PC], col = global pixel chunk (core-major)
            l_t = inp.tile([PIX_CHUNK, N_PC], f32)
            m_t = inp.tile([PIX_CHUNK, N_PC], f32)
            sky4_t = inp.tile([PIX_CHUNK, N_PC * 4], f16)
            for c in range(N_CORES):
                rows = slice(c * PIX_CHUNK, (c + 1) * PIX_CHUNK)
                nc.sync.dma_start(l_t[:, c * PCL:(c + 1) * PCL],
                                  lm_g[rows, 0:PCL])
                nc.sync.dma_start(m_t[:, c * PCL:(c + 1) * PCL],
                                  lm_g[rows, PCL:2 * PCL])
                nc.sync.dma_start(sky4_t[:, c * PCL * 4:(c + 1) * PCL * 4],
                                  sky_g[rows, :])

            # n1 = sqrt(1 - l^2 - m^2) - 1
            n1_t = inp.tile([PIX_CHUNK, N_PC], f32)
            sq_t = inp.tile([PIX_CHUNK, N_PC], f32)
            sq2_t = inp.tile([PIX_CHUNK, N_PC], f32)
            nc.vector.tensor_tensor(sq_t[:], l_t[:], l_t[:],
                                    op=AluOpType.mult)
            nc.vector.tensor_tensor(sq2_t[:], m_t[:], m_t[:],
                                    op=AluOpType.mult)
            nc.vector.tensor_tensor(sq_t[:], sq_t[:], sq2_t[:],
                                    op=AluOpType.add)
            nc.scalar.activation(
                sq2_t[:], sq_t[:], mybir.ActivationFunctionType.Sqrt,
                bias=1.0, scale=-1.0)
            nc.vector.tensor_scalar(
                n1_t[:], sq2_t[:], -1.0, None, op0=AluOpType.add)

            # u/v/w rows replicated across all 128 partitions
            reps = []
            for c in range(3):
                rep = inp.tile([128, VL], f32, tag=f"rep{c}")
                nc.sync.dma_start(rep[:], uvw_d[c:c + 1, :].to_broadcast(
                    (128, VL)))
                reps.append(rep)
            u_rep, v_rep, w_rep = reps

            vis_ps = vpsp.tile([36, VL], f32)

            with (
                tc.tile_pool(name="tx", bufs=2) as txp,
                tc.tile_pool(name="ty", bufs=2) as typ,
                tc.tile_pool(name="rt", bufs=2) as rp,
                tc.tile_pool(name="rat", bufs=2) as rap,
                tc.tile_pool(name="st", bufs=2) as sp,
                tc.tile_pool(name="ct", bufs=2) as cp,
            ):
             for _rep in range(repeat):
              for g in range(N_PC // GROUP):
                t_x = txp.tile([128, GFD], f32)
                t_y = typ.tile([128, GFD], f32)
                r_t = rp.tile([128, GFD], f32)
                ra_t = rap.tile([128, GFD], f32)
                s_t = sp.tile([128, GFD], f16)
                c_t = cp.tile([128, GFD], f16)

                for h in range(GROUP):
                    pc = g * GROUP + h
                    sl = slice(h * VL, (h + 1) * VL)
                    l_col = l_t[:, pc:pc + 1]
                    m_col = m_t[:, pc:pc + 1]
                    n1_col = n1_t[:, pc:pc + 1]
                    # t = l*u
                    nc.vector.tensor_scalar(
                        t_x[:, sl], u_rep[:], l_col, None,
                        op0=AluOpType.mult)
                    # t += m*v ; t += n1*w
                    nc.vector.affine_then_add(
                        t_y[:, sl], v_rep[:], t_x[:, sl],
                        scale=m_col, bias=0.0)
                    nc.vector.affine_then_add(
                        t_x[:, sl], w_rep[:], t_y[:, sl],
                        scale=n1_col, bias=0.0)

                # k = round(t); r = t - k; ra = |r|
                nc.vector.tensor_scalar(
                    t_y[:], t_x[:], MAGIC, MAGIC,
                    op0=AluOpType.add, op1=AluOpType.subtract)
                nc.vector.tensor_tensor(
                    r_t[:], t_x[:], t_y[:], op=AluOpType.subtract)
                nc.vector.tensor_scalar(
                    ra_t[:].bitcast(u32), r_t[:].bitcast(u32),
                    0x7FFFFFFF, None, op0=AluOpType.bitwise_and)

                nc.scalar.activation(
                    s_t[:], r_t[:], mybir.ActivationFunctionType.Sin,
                    bias=0.0, scale=TWO_PI)
                nc.scalar.activation(
                    c_t[:], ra_t[:], mybir.ActivationFunctionType.Sin,
                    bias=halfpi_t[:], scale=-TWO_PI)

                for h in range(GROUP):
                    pc = g * GROUP + h
                    sky_sl = sky4_t[:, pc * 4:(pc + 1) * 4]
                    start = pc == 0
                    stop = pc == N_PC - 1
                    for n in range(VL // MM_N):
                        vsl = slice(h * VL + n * MM_N, h * VL + (n + 1) * MM_N)
                        osl = slice(n * MM_N, (n + 1) * MM_N)
                        nc.tensor.matmul(
                            vis_ps[0:4, osl], sky_sl, s_t[:, vsl],
                            start=start, stop=stop, tile_position=(0, 0))
                        nc.tensor.matmul(
                            vis_ps[32:36, osl], sky_sl, c_t[:, vsl],
                            start=start, stop=stop, tile_position=(0, 32))

            # On-device combine. PSUM rows 0:4 = [SR0,SR1,SI0,SI1], rows
            # 32:36 = [CR0,CR1,CI0,CI1]; visR_b = CR_b + SI_b, visI_b =
            # CI_b - SR_b. Engine ops must start at a quadrant-aligned
            # partition base, so stage blocks at base 0 with DMAs (which
            # have no partition alignment rules) and fold the +/- into a
            # per-partition sign scalar: out = (S * sgn) + C.
            with tc.tile_pool(name="cmb", bufs=1) as cmbp:
                stage = cmbp.tile([36, VL], f32)
                c_blk = cmbp.tile([4, VL], f32)
                s_blk = cmbp.tile([4, VL], f32)
                sgn_p = cmbp.tile([2, 1], f32)
                sgn_n = cmbp.tile([2, 1], f32)
                sgn = cmbp.tile([4, 1], f32)
                out_t = cmbp.tile([4, VL], f16)
                nc.vector.memset(sgn_p[:], 1.0)
                nc.vector.memset(sgn_n[:], -1.0)
                nc.sync.dma_start(sgn[0:2, :], sgn_p[:])
                nc.sync.dma_start(sgn[2:4, :], sgn_n[:])
                nc.scalar.copy(stage[0:4, :], vis_ps[0:4, :])
                nc.scalar.copy(stage[32:36, :], vis_ps[32:36, :])
                nc.sync.dma_start(c_blk[:], stage[32:36, :])
                nc.sync.dma_start(s_blk[0:2, :], stage[2:4, :])  # SI0, SI1
                nc.sync.dma_start(s_blk[2:4, :], stage[0:2, :])  # SR0, SR1
                nc.vector.scalar_tensor_tensor(
                    out_t[:], s_blk[:], sgn[:], c_blk[:],
                    op0=AluOpType.mult, op1=AluOpType.add)
                nc.sync.dma_start(out_d[:], out_t[:])

    nc.compile()
    return nc


def _prep_inputs(sky_real, sky_imag, l_coords, m_coords, n_coords,
                 u_coords, v_coords, w_coords):
    # [N_PC, 128] views: chunk pc covers pixels pc*128 .. pc*128+127
    lc = l_coords.reshape(N_PC, PIX_CHUNK).astype(np.float32)
    mc = m_coords.reshape(N_PC, PIX_CHUNK).astype(np.float32)

    sky4 = np.stack([sky_real[0], sky_real[1], sky_imag[0], sky_imag[1]],
                    axis=1)                                       # [P, 4]
    sky4 = sky4.reshape(N_PC, PIX_CHUNK, 4).astype(np.float16)

    in_maps = []
    for c in range(N_CORES):
        pcs = slice(c * PCL, (c + 1) * PCL)
        # lm: [128, PCL(l) | PCL(m)]
        lm = np.concatenate([lc[pcs].T, mc[pcs].T], axis=1)
        lm = np.ascontiguousarray(lm, dtype=np.float32)
        # sky: [128, PCL*4], col j*4+k = chunk (c*PCL+j), quadrature k
        s4 = np.ascontiguousarray(
            sky4[pcs].transpose(1, 0, 2).reshape(PIX_CHUNK, PCL * 4))
        sl = slice(c * VL, (c + 1) * VL)
        uvw = np.ascontiguousarray(
            np.stack([u_coords[sl], v_coords[sl], w_coords[sl]])
            .astype(np.float32))
        in_maps.append({"lm": lm, "sky4": s4, "uvw": uvw})
    return in_maps


# Cached dispatch: run_bass_kernel_spmd (axon path -> run_bass_via_pjrt)
# rebuilds the jax.jit(shard_map(...)) closure on EVERY call, retracing and
# relowering the whole program each time -- hundreds of ms of pure host
# overhead per call. Build the jitted executable once and reuse it.
_RUNNER = None


def _get_runner():
    global _RUNNER
    if _RUNNER is not None:
        return _RUNNER

    import jax
    import concourse.mybir as mybir
    from concourse import bass2jax
    from jax.experimental.shard_map import shard_map
    from jax.sharding import Mesh, PartitionSpec

    nc = _build()
    bass2jax.install_neuronx_cc_hook()
    assert nc.dbg_addr is None
    part_name = (nc.partition_id_tensor.name
                 if nc.partition_id_tensor else None)

    in_names, out_names, out_avals = [], [], []
    for alloc in nc.m.functions[0].allocations:
        if not isinstance(alloc, mybir.MemoryLocationSet):
            continue
        name = alloc.memorylocations[0].name
        if alloc.kind == "ExternalInput":
            if name != part_name:
                in_names.append(name)
        elif alloc.kind == "ExternalOutput":
            shape = tuple(alloc.tensor_shape)
            dtype = mybir.dt.np(alloc.dtype)
            out_names.append(name)
            out_avals.append(jax.core.ShapedArray(shape, dtype))
    n_params = len(in_names)
    # Outputs are fully written by the kernel, so no donated zero output
    # buffers are passed (saves their upload).
    all_names = list(in_names)
    if part_name is not None:
        all_names.append(part_name)
    all_names = tuple(all_names)

    def _body(*args):
        operands = list(args)
        if part_name is not None:
            operands.append(bass2jax.partition_id_tensor())
        outs = bass2jax._bass_exec_p.bind(
            *operands,
            out_avals=tuple(out_avals),
            in_names=all_names,
            out_names=tuple(out_names),
            lowering_input_output_aliases=(),
            sim_require_finite=True,
            sim_require_nnan=True,
            nc=nc,
        )
        return tuple(outs)

    devices = jax.devices()[:N_CORES]
    mesh = Mesh(np.asarray(devices), ("core",))
    n_outs = len(out_names)
    sharded = jax.jit(
        shard_map(_body, mesh=mesh,
                  in_specs=(PartitionSpec("core"),) * n_params,
                  out_specs=(PartitionSpec("core"),) * n_outs,
                  check_rep=False),
        keep_unused=True,
    )
    _RUNNER = (sharded, in_names, out_names, out_avals)
    return _RUNNER


def kernel(sky_real, sky_imag, l_coords, m_coords, n_coords,
           u_coords, v_coords, w_coords):
    sharded, in_names, out_names, out_avals = _get_runner()

    in_maps = _prep_inputs(sky_real, sky_imag, l_coords, m_coords, n_coords,
                           u_coords, v_coords, w_coords)
    concat_in = [
        np.concatenate([in_maps[c][name] for c in range(N_CORES)], axis=0)
        for name in in_names
    ]
    out_arrs = sharded(*concat_in)
    o4 = np.asarray(out_arrs[out_names.index("out4")]).astype(np.float32)
    o4 = o4.reshape(N_CORES, 4, VL)

    vis = np.empty((B, V), dtype=np.complex64)
    for c in range(N_CORES):
        sl = slice(c * VL, (c + 1) * VL)
        vr0, vr1, vi0, vi1 = o4[c]
        vis[0, sl] = vr0 + 1j * vi0
        vis[1, sl] = vr1 + 1j * vi1
    return vis


# revision 11
# speedup vs baseline: 6.7947x; 1.0110x over previous
"""Chunked non-uniform DFT on 8 Trainium2 NeuronCores (Bass/Tile).

vis[b,k] = sum_p exp(-2pi*i*(u_k*l_p + v_k*m_p + w_k*(n_p-1))) * sky[b,p]

The wall-clock through the axon PJRT tunnel is dominated by a fixed
round-trip (~60ms) plus wire bytes at ~40MB/s, so the kernel minimizes
host<->device traffic:
  - Each core uploads only 1/8 of the pixel arrays (its l/m blocks and
    sky quadrature columns) plus its own uvw slice (~56KB/core); the
    full pixel arrays are reassembled on-device with a DRAM AllGather
    over NeuronLink.
  - n-1 is recomputed on device from l,m (n = sqrt(1-l^2-m^2)), saving
    one upload array.
  - Partial sums are combined on device (visR_b = C.R_b + S.I_b,
    visI_b = C.I_b - S.R_b) and downloaded as fp16 [4, VL] (16KB/core).
  - No donated zero output buffers (kernel fully writes its output).
  - The jitted shard_map executable is built once and cached; the stock
    run_bass_kernel_spmd re-jits on every call.

Compute per core (visibilities sharded 8 ways => V_local = 2048):
  - t[p,k] = l_p*u_k + m_p*v_k + (n_p-1)*w_k on the Vector engine:
    u/v/w replicated across 128 partitions once (broadcast DMA); per
    pixel-chunk, l/m/n1 enter as per-partition scalars via
    tensor_scalar + 2x affine_then_add.
  - r = t - round(t) in [-0.5, 0.5] via magic-number round + subtract.
  - S = sin(2*pi*r), C = sin(pi/2 - 2*pi*|r|) = cos(2*pi*t) on ACT, fp16.
  - PE matmuls: sky (4 columns R0,R1,I0,I1 fp16) stationary, S/C moving,
    col groups 0 / 32 accumulating into PSUM rows 0-3 / 32-35.
"""

import numpy as np

B = 2
P = 16384
V = 16384
N_CORES = 8
VL = V // N_CORES  # 2048

MAGIC = float(1.5 * 2**23)
TWO_PI = float(2.0 * np.pi)
HALF_PI = float(0.5 * np.pi)

PIX_CHUNK = 128
N_PC = P // PIX_CHUNK        # 128 pixel chunks total
PCL = N_PC // N_CORES        # 16 pixel chunks owned per core
GROUP = 2                    # pix-chunks per batched round/abs/ACT group
MM_N = 512                   # stage-C matmul free dim (one PSUM bank)

RG = [list(range(N_CORES))]


def _build(repeat=1):
    import concourse.bacc as bacc
    import concourse.mybir as mybir
    import concourse.tile as tile
    from concourse.alu_op_type import AluOpType

    nc = bacc.Bacc("TRN2", target_bir_lowering=False, debug=False,
                   num_devices=N_CORES)
    f32 = mybir.dt.float32
    f16 = mybir.dt.float16
    u32 = mybir.dt.uint32

    # Per-core shards: l/m blocks for this core's PCL pixel chunks,
    # sky quadrature (R0,R1,I0,I1 per chunk), own uvw slice.
    lm_d = nc.dram_tensor("lm", [PIX_CHUNK, PCL * 2], f32,
                          kind="ExternalInput")
    sky_d = nc.dram_tensor("sky4", [PIX_CHUNK, PCL * 4], f16,
                           kind="ExternalInput")
    uvw_d = nc.dram_tensor("uvw", [3, VL], f32, kind="ExternalInput")
    out_d = nc.dram_tensor("out4", [4, VL], f16, kind="ExternalOutput")

    GFD = GROUP * VL

    with tile.TileContext(nc) as tc:
        with (
            tc.tile_pool(name="dram", bufs=1, space="DRAM") as dramp,
            tc.tile_pool(name="const", bufs=1) as constp,
            tc.tile_pool(name="inp", bufs=1) as inp,
            tc.tile_pool(name="vps", bufs=1, space="PSUM") as vpsp,
        ):
            # ---- gather the pixel arrays from all cores over NeuronLink
            lm_bnc = dramp.tile([PIX_CHUNK, PCL * 2], f32)
            sky_bnc = dramp.tile([PIX_CHUNK, PCL * 4], f16)
            lm_g = dramp.tile([N_CORES * PIX_CHUNK, PCL * 2], f32,
                              addr_space="Shared")
            sky_g = dramp.tile([N_CORES * PIX_CHUNK, PCL * 4], f16,
                               addr_space="Shared")
            nc.gpsimd.dma_start(lm_bnc[:], lm_d[:])
            nc.gpsimd.dma_start(sky_bnc[:], sky_d[:])
            nc.gpsimd.collective_compute(
                "AllGather", mybir.AluOpType.bypass, replica_groups=RG,
                ins=[lm_bnc.opt()], outs=[lm_g.opt()])
            nc.gpsimd.collective_compute(
                "AllGather", mybir.AluOpType.bypass, replica_groups=RG,
                ins=[sky_bnc.opt()], outs=[sky_g.opt()])

            halfpi_t = constp.tile([128, 1], f32)
            nc.vector.memset(halfpi_t[:], HALF_PI)

            # l_t/m_t: [128, N_PC], col = global pixel chunk (core-major)
            l_t = inp.tile([PIX_CHUNK, N_PC], f32)
            m_t = inp.tile([PIX_CHUNK, N_PC], f32)
            sky4_t = inp.tile([PIX_CHUNK, N_PC * 4], f16)
            for c in range(N_CORES):
                rows = slice(c * PIX_CHUNK, (c + 1) * PIX_CHUNK)
                nc.sync.dma_start(l_t[:, c * PCL:(c + 1) * PCL],
                                  lm_g[rows, 0:PCL])
                nc.sync.dma_start(m_t[:, c * PCL:(c + 1) * PCL],
                                  lm_g[rows, PCL:2 * PCL])
                nc.sync.dma_start(sky4_t[:, c * PCL * 4:(c + 1) * PCL * 4],
                                  sky_g[rows, :])

            # n1 = sqrt(1 - l^2 - m^2) - 1
            n1_t = inp.tile([PIX_CHUNK, N_PC], f32)
            sq_t = inp.tile([PIX_CHUNK, N_PC], f32)
            sq2_t = inp.tile([PIX_CHUNK, N_PC], f32)
            nc.vector.tensor_tensor(sq_t[:], l_t[:], l_t[:],
                                    op=AluOpType.mult)
            nc.vector.tensor_tensor(sq2_t[:], m_t[:], m_t[:],
                                    op=AluOpType.mult)
            nc.vector.tensor_tensor(sq_t[:], sq_t[:], sq2_t[:],
                                    op=AluOpType.add)
            nc.scalar.activation(
                sq2_t[:], sq_t[:], mybir.ActivationFunctionType.Sqrt,
                bias=1.0, scale=-1.0)
            nc.vector.tensor_scalar(
                n1_t[:], sq2_t[:], -1.0, None, op0=AluOpType.add)

            # u/v/w rows replicated across all 128 partitions
            reps = []
            for c in range(3):
                rep = inp.tile([128, VL], f32, tag=f"rep{c}")
                nc.sync.dma_start(rep[:], uvw_d[c:c + 1, :].to_broadcast(
                    (128, VL)))
                reps.append(rep)
            u_rep, v_rep, w_rep = reps

            vis_ps = vpsp.tile([36, VL], f32)

            with (
                tc.tile_pool(name="tx", bufs=2) as txp,
                tc.tile_pool(name="ty", bufs=2) as typ,
                tc.tile_pool(name="rt", bufs=2) as rp,
                tc.tile_pool(name="rat", bufs=2) as rap,
                tc.tile_pool(name="st", bufs=2) as sp,
                tc.tile_pool(name="ct", bufs=2) as cp,
            ):
             for _rep in range(repeat):
              for g in range(N_PC // GROUP):
                t_x = txp.tile([128, GFD], f32)
                t_y = typ.tile([128, GFD], f32)
                r_t = rp.tile([128, GFD], f32)
                ra_t = rap.tile([128, GFD], f32)
                s_t = sp.tile([128, GFD], f16)
                c_t = cp.tile([128, GFD], f16)

                for h in range(GROUP):
                    pc = g * GROUP + h
                    sl = slice(h * VL, (h + 1) * VL)
                    l_col = l_t[:, pc:pc + 1]
                    m_col = m_t[:, pc:pc + 1]
                    n1_col = n1_t[:, pc:pc + 1]
                    # t = l*u
                    nc.vector.tensor_scalar(
                        t_x[:, sl], u_rep[:], l_col, None,
                        op0=AluOpType.mult)
                    # t += m*v ; t += n1*w
                    nc.vector.affine_then_add(
                        t_y[:, sl], v_rep[:], t_x[:, sl],
                        scale=m_col, bias=0.0)
                    nc.vector.affine_then_add(
                        t_x[:, sl], w_rep[:], t_y[:, sl],
                        scale=n1_col, bias=0.0)

                # k = round(t); r = t - k; ra = |r|
                nc.vector.tensor_scalar(
                    t_y[:], t_x[:], MAGIC, MAGIC,
                    op0=AluOpType.add, op1=AluOpType.subtract)
                nc.vector.tensor_tensor(
                    r_t[:], t_x[:], t_y[:], op=AluOpType.subtract)
                nc.vector.tensor_scalar(
                    ra_t[:].bitcast(u32), r_t[:].bitcast(u32),
                    0x7FFFFFFF, None, op0=AluOpType.bitwise_and)

                nc.scalar.activation(
                    s_t[:], r_t[:], mybir.ActivationFunctionType.Sin,
                    bias=0.0, scale=TWO_PI)
                nc.scalar.activation(
                    c_t[:], ra_t[:], mybir.ActivationFunctionType.Sin,
                    bias=halfpi_t[:], scale=-TWO_PI)

                for h in range(GROUP):
                    pc = g * GROUP + h
                    sky_sl = sky4_t[:, pc * 4:(pc + 1) * 4]
                    start = pc == 0
                    stop = pc == N_PC - 1
                    for n in range(VL // MM_N):
                        vsl = slice(h * VL + n * MM_N, h * VL + (n + 1) * MM_N)
                        osl = slice(n * MM_N, (n + 1) * MM_N)
                        nc.tensor.matmul(
                            vis_ps[0:4, osl], sky_sl, s_t[:, vsl],
                            start=start, stop=stop, tile_position=(0, 0))
                        nc.tensor.matmul(
                            vis_ps[32:36, osl], sky_sl, c_t[:, vsl],
                            start=start, stop=stop, tile_position=(0, 32))

            # On-device combine. PSUM rows 0:4 = [SR0,SR1,SI0,SI1], rows
            # 32:36 = [CR0,CR1,CI0,CI1]; visR_b = CR_b + SI_b, visI_b =
            # CI_b - SR_b. Engine ops must start at a quadrant-aligned
            # partition base, so stage blocks at base 0 with DMAs (which
            # have no partition alignment rules) and fold the +/- into a
            # per-partition sign scalar: out = (S * sgn) + C.
            with tc.tile_pool(name="cmb", bufs=1) as cmbp:
                stage = cmbp.tile([36, VL], f32)
                c_blk = cmbp.tile([4, VL], f32)
                s_blk = cmbp.tile([4, VL], f32)
                sgn_p = cmbp.tile([2, 1], f32)
                sgn_n = cmbp.tile([2, 1], f32)
                sgn = cmbp.tile([4, 1], f32)
                out_t = cmbp.tile([4, VL], f16)
                nc.vector.memset(sgn_p[:], 1.0)
                nc.vector.memset(sgn_n[:], -1.0)
                nc.sync.dma_start(sgn[0:2, :], sgn_p[:])
                nc.sync.dma_start(sgn[2:4, :], sgn_n[:])
                nc.scalar.copy(stage[0:4, :], vis_ps[0:4, :])
                nc.scalar.copy(stage[32:36, :], vis_ps[32:36, :])
                nc.sync.dma_start(c_blk[:], stage[32:36, :])
                nc.sync.dma_start(s_blk[0:2, :], stage[2:4, :])  # SI0, SI1
                nc.sync.dma_start(s_blk[2:4, :], stage[0:2, :])  # SR0, SR1
                nc.vector.scalar_tensor_tensor(
                    out_t[:], s_blk[:], sgn[:], c_blk[:],
                    op0=AluOpType.mult, op1=AluOpType.add)
                nc.sync.dma_start(out_d[:], out_t[:])

    nc.compile()
    return nc


def _prep_inputs(sky_real, sky_imag, l_coords, m_coords, n_coords,
                 u_coords, v_coords, w_coords):
    """Concatenated (axis 0, core-major) per-core shards, keyed by input
    tensor name. Chunk pc covers pixels pc*128 .. pc*128+127; core c owns
    chunks c*PCL .. (c+1)*PCL-1."""
    lc = l_coords.reshape(N_CORES, PCL, PIX_CHUNK).transpose(0, 2, 1)
    mc = m_coords.reshape(N_CORES, PCL, PIX_CHUNK).transpose(0, 2, 1)
    lm = np.concatenate([lc, mc], axis=2).astype(np.float32, copy=False)
    lm = np.ascontiguousarray(lm).reshape(N_CORES * PIX_CHUNK, PCL * 2)

    sky4 = np.stack([sky_real[0], sky_real[1], sky_imag[0], sky_imag[1]],
                    axis=1).astype(np.float16)                    # [P, 4]
    sky4 = np.ascontiguousarray(
        sky4.reshape(N_CORES, PCL, PIX_CHUNK, 4).transpose(0, 2, 1, 3)
    ).reshape(N_CORES * PIX_CHUNK, PCL * 4)

    uvw = np.ascontiguousarray(
        np.stack([u_coords.reshape(N_CORES, VL),
                  v_coords.reshape(N_CORES, VL),
                  w_coords.reshape(N_CORES, VL)], axis=1)
        .astype(np.float32, copy=False)).reshape(N_CORES * 3, VL)

    return {"lm": lm, "sky4": sky4, "uvw": uvw}


# Cached dispatch: run_bass_kernel_spmd (axon path -> run_bass_via_pjrt)
# rebuilds the jax.jit(shard_map(...)) closure on EVERY call, retracing and
# relowering the whole program each time -- hundreds of ms of pure host
# overhead per call. Build the jitted executable once and reuse it.
_RUNNER = None


def _get_runner():
    global _RUNNER
    if _RUNNER is not None:
        return _RUNNER

    import jax
    import concourse.mybir as mybir
    from concourse import bass2jax
    from jax.experimental.shard_map import shard_map
    from jax.sharding import Mesh, PartitionSpec

    nc = _build()
    bass2jax.install_neuronx_cc_hook()
    assert nc.dbg_addr is None
    part_name = (nc.partition_id_tensor.name
                 if nc.partition_id_tensor else None)

    in_names, out_names, out_avals = [], [], []
    for alloc in nc.m.functions[0].allocations:
        if not isinstance(alloc, mybir.MemoryLocationSet):
            continue
        name = alloc.memorylocations[0].name
        if alloc.kind == "ExternalInput":
            if name != part_name:
                in_names.append(name)
        elif alloc.kind == "ExternalOutput":
            shape = tuple(alloc.tensor_shape)
            dtype = mybir.dt.np(alloc.dtype)
            out_names.append(name)
            out_avals.append(jax.core.ShapedArray(shape, dtype))
    n_params = len(in_names)
    # Outputs are fully written by the kernel, so no donated zero output
    # buffers are passed (saves their upload).
    all_names = list(in_names)
    if part_name is not None:
        all_names.append(part_name)
    all_names = tuple(all_names)

    def _body(*args):
        operands = list(args)
        if part_name is not None:
            operands.append(bass2jax.partition_id_tensor())
        outs = bass2jax._bass_exec_p.bind(
            *operands,
            out_avals=tuple(out_avals),
            in_names=all_names,
            out_names=tuple(out_names),
            lowering_input_output_aliases=(),
            sim_require_finite=True,
            sim_require_nnan=True,
            nc=nc,
        )
        return tuple(outs)

    devices = jax.devices()[:N_CORES]
    mesh = Mesh(np.asarray(devices), ("core",))
    n_outs = len(out_names)

    global_avals = {}
    for alloc in nc.m.functions[0].allocations:
        if not isinstance(alloc, mybir.MemoryLocationSet):
            continue
        name = alloc.memorylocations[0].name
        if alloc.kind == "ExternalInput" and name != part_name:
            shape = tuple(alloc.tensor_shape)
            global_avals[name] = jax.ShapeDtypeStruct(
                (N_CORES * shape[0],) + shape[1:], mybir.dt.np(alloc.dtype))
    arg_structs = [global_avals[name] for name in in_names]

    def compile_fn():
        jitted = jax.jit(
            shard_map(_body, mesh=mesh,
                      in_specs=(PartitionSpec("core"),) * n_params,
                      out_specs=(PartitionSpec("core"),) * n_outs,
                      check_rep=False),
            keep_unused=True,
        )
        return jitted.lower(*arg_structs).compile()

    sharded = bass2jax.fast_dispatch_compile(compile_fn)
    _RUNNER = (sharded, in_names, out_names, out_avals)
    return _RUNNER


def kernel(sky_real, sky_imag, l_coords, m_coords, n_coords,
           u_coords, v_coords, w_coords):
    sharded, in_names, out_names, out_avals = _get_runner()

    ins = _prep_inputs(sky_real, sky_imag, l_coords, m_coords, n_coords,
                       u_coords, v_coords, w_coords)
    out_arrs = sharded(*[ins[name] for name in in_names])
    o4 = np.asarray(out_arrs[out_names.index("out4")]).astype(np.float32)
    o4 = o4.reshape(N_CORES, 4, VL)

    vis = np.empty((B, V), dtype=np.complex64)
    for c in range(N_CORES):
        sl = slice(c * VL, (c + 1) * VL)
        vr0, vr1, vi0, vi1 = o4[c]
        vis[0, sl] = vr0 + 1j * vi0
        vis[1, sl] = vr1 + 1j * vi1
    return vis
